# revision 20
# baseline (speedup 1.0000x reference)
"""Trainium2 Bass kernel for nn_Decoder_43336220016932.

Luong-attention LSTM decoder with teacher forcing:
  out[b,t,:] = log_softmax(tanh([ctx_t, h_t] @ W_fc + b_fc))

Strategy (8 NeuronCores), v3:
  - Vocab-sharded tensor parallel: core i owns W_fc[:, i*4000:(i+1)*4000]
    resident in SBUF as fp8(e4m3), and the big FC runs in DoubleRow
    perf mode (2 fp8 weights per PE cell, 256-deep contraction per
    pass) against fp8 copies of ctx/h. The serial LSTM recurrence is
    replicated on all cores (latency-bound; replication is free).
  - Recurrence keeps all state transposed ([U -> partitions, B -> free]).
    Gate pre-activations accumulate entirely in PSUM: z_x (DoubleRow
    precompute, fp8 in SBUF, b_lstm folded in) is injected with one
    identity matmul per gate, W_h matmuls accumulate on top, and the
    gate nonlinearity reads PSUM directly. Separate PSUM tile per gate
    so gates never false-serialize.
  - Attention (scores/softmax/ctx) batched over the 63 steps after the
    recurrence, softmax ops merged across batch-pair halves.
  - log_softmax: logits are tanh-bounded, so sumexp uses a fixed shift
    exp(x-1). The vocab-shard merge is two pipelined AllReduce(add)
    calls (chunks 0-11 overlap the remaining FC; chunks 12-15 expose
    only one collective latency). Logits stay resident in SBUF as
    fp8(e3m4); the finalize is a per-partition bias add split across
    the Scalar and Vector engines, streaming bf16 straight to HBM.
    No DRAM round-trips anywhere.
"""
from contextlib import ExitStack

import numpy as np
import ml_dtypes

import concourse.bass as bass
import concourse.tile as tile
from concourse import bacc, mybir
from concourse.bass_utils import run_bass_kernel_spmd
from concourse.masks import make_identity

B, S, L, U, E, V = 32, 64, 64, 512, 256, 32000
T = S - 1                  # 63 decode steps
NC = 8                     # cores
VS = V // NC               # 4000 vocab shard per core
TB = T * B                 # 2016 (t, b) rows, t-major
G4U = 4 * U                # 2048
BL = B * L                 # 2048
AF = mybir.ActivationFunctionType
ALU = mybir.AluOpType
AX = mybir.AxisListType
PM = mybir.MatmulPerfMode
F32 = mybir.dt.float32
BF16 = mybir.dt.bfloat16
FP8 = mybir.dt.float8e4    # e4m3 (matmul DoubleRow capable)
FP8L = mybir.dt.float8e3   # e3m4: best near +-1 for tanh-bounded logits
I32 = mybir.dt.int32

_CACHE = {}


def build(n_cores=NC):
    """Build the SPMD Bass program (same program on every core)."""
    nc = bacc.Bacc("TRN2", target_bir_lowering=False, debug=False,
                   num_devices=n_cores)

    # ---- external I/O ----
    tidx = nc.dram_tensor("tidx", [TB, 1], I32, kind="ExternalInput").ap()
    emb_bf = nc.dram_tensor("emb_bf", [V, E], BF16, kind="ExternalInput").ap()
    wx8_d = nc.dram_tensor("wx8", [128, 2 * G4U], FP8, kind="ExternalInput").ap()
    blp_d = nc.dram_tensor("blp", [128, 16], F32, kind="ExternalInput").ap()
    wh_bf = nc.dram_tensor("wh_bf", [U, G4U], BF16, kind="ExternalInput").ap()
    enc_bf = nc.dram_tensor("enc_bf", [B, L, U], BF16, kind="ExternalInput").ap()
    enct_bf = nc.dram_tensor("enct_bf", [U, BL], BF16, kind="ExternalInput").ap()
    wa_bf = nc.dram_tensor("wa_bf", [U, U], BF16, kind="ExternalInput").ap()
    h0 = nc.dram_tensor("h0", [B, U], F32, kind="ExternalInput").ap()
    c0 = nc.dram_tensor("c0", [B, U], F32, kind="ExternalInput").ap()
    wfc8_d = nc.dram_tensor("wfc8", [128, 8 * VS], FP8, kind="ExternalInput").ap()
    bfc_bf = nc.dram_tensor("bfc_bf", [1, VS], BF16, kind="ExternalInput").ap()
    out = nc.dram_tensor("out", [B, T, VS], BF16, kind="ExternalOutput").ap()

    GA = 12                  # AllReduce group A: chunks 0..11
    with tile.TileContext(nc) as tc, ExitStack() as perm:
        # ---------------- permanent pools ----------------
        konst = perm.enter_context(tc.tile_pool(name="konst", bufs=1))
        wpool = perm.enter_context(tc.tile_pool(name="wpool", bufs=1))
        hpool = perm.enter_context(tc.tile_pool(name="hpool", bufs=1))
        dram = perm.enter_context(tc.tile_pool(name="dram", bufs=1, space="DRAM"))
        stats = perm.enter_context(tc.tile_pool(name="stats", bufs=1))

        idt = konst.tile([128, 128], BF16)
        make_identity(nc, idt[:])
        idt8 = konst.tile([128, 128], FP8)
        make_identity(nc, idt8[:])
        negone = konst.tile([128, 1], F32)
        nc.vector.memset(negone[:], -1.0)
        idtf = konst.tile([128, 128], F32)
        make_identity(nc, idtf[:])

        # H: h.T history (bf16, for recurrence + scores).
        # col = k*2048 + slot*32 + b; slot 0 = h_init, slot t+1 = after step t
        H = hpool.tile([128, 4 * 64 * B], BF16)
        Hk = H[:].rearrange("p (k s b) -> p k s b", k=4, s=64)
        # fp8 twins for the DoubleRow FC: pair-major views (kp, k2)
        H8 = hpool.tile([128, 4 * 64 * B], FP8)
        H8v = H8[:].rearrange("p (kp k2 r) -> p kp k2 r", kp=2, k2=2)
        Gc8 = hpool.tile([128, 4 * TB], FP8)     # ctx.T, col = k*2016+t*32+b
        Gc8v = Gc8[:].rearrange("p (kp k2 r) -> p kp k2 r", kp=2, k2=2)
        Gc8k = Gc8[:].rearrange("p (k t b) -> p k t b", k=4, t=T)
        cT = hpool.tile([128, 128], F32)         # c.T state, col = k*32+b

        # resident: W_fc shard (fp8, pair-interleaved [p, kp, k2, v])
        wfc_sb = wpool.tile([128, 8 * VS], FP8)
        wfcv = wfc_sb[:].rearrange("p (kp k2 v) -> p kp k2 v", kp=4, k2=2)
        bfc_bc = wpool.tile([128, VS], BF16)
        blp_sb = wpool.tile([128, 16], F32)

        # per-row ((t,b) grouped [128 x 16]) log-softmax stats
        lsum_sb = stats.tile([128, 16], F32)   # local sum exp(x - 1)
        sg_sb = stats.tile([128, 16], F32)     # global sum
        nlz_sb = stats.tile([128, 16], F32)    # -(1 + ln(global sum))

        # DRAM scratch (collectives only) — one contiguous pair per group
        ccs_in = {0: dram.tile([128, 12], F32, name="ccs_in_a"),
                  1: dram.tile([128, 4], F32, name="ccs_in_b")}
        ccs_out = {0: dram.tile([128, 12], F32, name="ccs_out_a"),
                   1: dram.tile([128, 4], F32, name="ccs_out_b")}

        # enc/attention inputs live P..D1 in their own space so their DMAs
        # and the Wa projection can run during the recurrence (a pool that
        # reuses R-phase SBUF would have to wait for R to finish).
        encs = ExitStack()
        enc_pool = encs.enter_context(tc.tile_pool(name="encs", bufs=1))
        enct_sb = enc_pool.tile([128, 4 * BL], BF16)
        for k in range(4):
            nc.sync.dma_start(enct_sb[:, k * BL:(k + 1) * BL],
                              enct_bf[k * 128:(k + 1) * 128, :])
        wa_sb = enc_pool.tile([128, 4 * U], BF16)   # [k, k*512 + m]
        for k in range(4):
            nc.sync.dma_start(wa_sb[:, k * U:(k + 1) * U],
                              wa_bf[k * 128:(k + 1) * 128, :])
        enc_sb = enc_pool.tile([128, 16 * U], BF16)  # 2 b per tilegroup
        for j in range(16):
            nc.sync.dma_start(enc_sb[0:64, j * U:(j + 1) * U],
                              enc_bf[2 * j, :, :])
            nc.sync.dma_start(enc_sb[64:128, j * U:(j + 1) * U],
                              enc_bf[2 * j + 1, :, :])
        epT_sb = enc_pool.tile([128, 4 * BL], BF16)  # ep.T [u-chunk,(b,l)]

        with ExitStack() as mid:
            mwp = mid.enter_context(tc.tile_pool(name="midw", bufs=1))
            # z_x.T for every step, fp8 e4m3: col = mi*2016 + t*32 + b
            zxT = mwp.tile([128, 16 * TB], FP8)
            zxv = zxT[:].rearrange("p (m t b) -> p m t b", m=16, t=T)
            wh_sb = mwp.tile([128, 4 * G4U], BF16)      # [k, k*2048 + m]

            # ================= phase P: precompute =================
            with ExitStack() as pp:
                psb = pp.enter_context(tc.tile_pool(name="p_sbuf", bufs=4))
                pps = pp.enter_context(
                    tc.tile_pool(name="p_psum", bufs=2, space="PSUM"))
                pone = pp.enter_context(tc.tile_pool(name="p_one", bufs=1))

                # --- embedding gather (issued first) ---
                ixa = pone.tile([128, 16], I32)
                for i in range(16):
                    r0 = i * 128
                    rows = min(128, TB - r0)
                    nc.sync.dma_start(ixa[:rows, i:i + 1], tidx[r0:r0 + rows, :])
                xgs = []
                for i in range(16):
                    r0 = i * 128
                    rows = min(128, TB - r0)
                    xg = psb.tile([128, E], BF16, tag="xg")
                    nc.gpsimd.indirect_dma_start(
                        out=xg[:rows, :], out_offset=None,
                        in_=emb_bf[:],
                        in_offset=bass.IndirectOffsetOnAxis(ap=ixa[:rows, i:i + 1],
                                                            axis=0),
                    )
                    xgs.append(xg)

                # --- weight / input DMAs (no deps; start early) ---
                for k in range(8):
                    nc.sync.dma_start(wfc_sb[:, k * VS:(k + 1) * VS],
                                      wfc8_d[:, k * VS:(k + 1) * VS])
                nc.sync.dma_start(bfc_bc[:], bfc_bf.to_broadcast([128, VS]))
                nc.sync.dma_start(blp_sb[:], blp_d[:, :])
                wx8_sb = pone.tile([128, 2 * G4U], FP8)
                nc.sync.dma_start(wx8_sb[:], wx8_d[:, :])
                wx8v = wx8_sb[:].rearrange("p (e m) -> p e m", e=2)
                for k in range(4):
                    nc.sync.dma_start(wh_sb[:, k * G4U:(k + 1) * G4U],
                                      wh_bf[k * 128:(k + 1) * 128, :])

                # --- transpose gathers -> XT directly in fp8 ---
                xt8 = pone.tile([128, 2 * TB], FP8)
                for i in range(16):
                    r0 = i * 128
                    rows = min(128, TB - r0)
                    for cc in range(2):
                        tp = pps.tile([128, 128], BF16, tag="tpb")
                        nc.tensor.transpose(tp[:, :rows],
                                            xgs[i][:rows, cc * 128:(cc + 1) * 128],
                                            idt[:rows, :rows])
                        nc.vector.tensor_copy(
                            xt8[:, cc * TB + r0: cc * TB + r0 + rows],
                            tp[:, :rows])
                xt8v = xt8[:].rearrange("p (e c) -> p e c", e=2)

                # --- h0/c0 transposed init ---
                hc_sb = psb.tile([B, U], F32, tag="hc")
                nc.sync.dma_start(hc_sb[:, :], h0[:, :])
                cc_sb = psb.tile([B, U], F32, tag="hc2")
                nc.sync.dma_start(cc_sb[:, :], c0[:, :])
                for k in range(4):
                    tp = pps.tile([128, 128], F32, tag="tp")
                    nc.tensor.transpose(tp[:, :B],
                                        hc_sb[:B, k * 128:(k + 1) * 128],
                                        idtf[:B, :B])
                    nc.vector.tensor_copy(Hk[:, k, 0, :], tp[:, :B])
                    tp2 = pps.tile([128, 128], F32, tag="tp")
                    nc.tensor.transpose(tp2[:, :B],
                                        cc_sb[:B, k * 128:(k + 1) * 128],
                                        idtf[:B, :B])
                    nc.vector.tensor_copy(cT[:, k * B:(k + 1) * B], tp2[:, :B])

                # --- Zx.T = W_x.T @ X.T (DoubleRow), + b_lstm on cast ---
                for mi in range(16):
                    for nb in range(4):
                        t0 = nb * 16
                        tn = min(16, T - t0)
                        ncols = tn * B
                        zps = pps.tile([128, 512], F32, tag="zx")
                        nc.tensor.matmul(
                            zps[:, :ncols],
                            wx8v[:, :, mi * 128:(mi + 1) * 128],
                            xt8v[:, :, t0 * B: t0 * B + ncols],
                            start=True, stop=True, perf_mode=PM.DoubleRow)
                        nc.vector.tensor_scalar_add(
                            zxT[:, mi * TB + t0 * B: mi * TB + t0 * B + ncols],
                            zps[:, :ncols], blp_sb[:, mi:mi + 1])

                # --- ep.T = (enc @ Wa).T (runs during P; feeds D1) ---
                for mu in range(4):
                    for nb in range(4):
                        eps_ = pps.tile([128, 512], F32, tag="zx")
                        for k in range(4):
                            nc.tensor.matmul(
                                eps_[:, :],
                                wa_sb[:, k * U + mu * 128:
                                      k * U + (mu + 1) * 128],
                                enct_sb[:, k * BL + nb * 512:
                                        k * BL + (nb + 1) * 512],
                                start=(k == 0), stop=(k == 3))
                        nc.vector.tensor_copy(
                            epT_sb[:, mu * BL + nb * 512:
                                   mu * BL + (nb + 1) * 512],
                            eps_[:])

            # ================= phase R: recurrence =================
            # Gate layout host-permuted to [g, i, f, o]. Per gate: one
            # identity matmul injects z_x into PSUM, W_h matmuls
            # accumulate, the activation reads PSUM directly.
            with ExitStack() as rr:
                rps = rr.enter_context(
                    tc.tile_pool(name="r_psum", bufs=2, space="PSUM"))
                rga = rr.enter_context(tc.tile_pool(name="r_gate", bufs=2))

                c_prev = cT
                for t in range(T):
                    gate = {}
                    pg = {}
                    # inject z_x for all 4 gates first (no dep on h_t)
                    for gi in range(4):
                        pg[gi] = rps.tile([128, 128], F32, tag=f"pg{gi}",
                                          name=f"pg{gi}")
                        nc.tensor.matmul(
                            pg[gi][:].rearrange("p (m b) -> p m b", m=4),
                            idt8[:, :],
                            zxv[:, 4 * gi:4 * gi + 4, t, :],
                            start=True, stop=False)
                    for gi, fn in enumerate((AF.Tanh, AF.Sigmoid,
                                             AF.Sigmoid, AF.Sigmoid)):
                        for m2 in range(4):
                            mi = gi * 4 + m2
                            for k in range(4):
                                nc.tensor.matmul(
                                    pg[gi][:, m2 * B:(m2 + 1) * B],
                                    wh_sb[:, k * G4U + mi * 128:
                                          k * G4U + (mi + 1) * 128],
                                    Hk[:, k, t, :],
                                    start=False, stop=(k == 3))
                        gt = rga.tile([128, 128], F32, tag=f"g{gi}",
                                      name=f"g{gi}")
                        nc.scalar.activation(gt[:], pg[gi][:], fn)
                        gate[gi] = gt
                        if gi == 1:      # i ready: i*tanh(g)
                            ig = rga.tile([128, 128], F32, tag="ig")
                            nc.vector.tensor_mul(ig[:], gate[1][:], gate[0][:])
                        elif gi == 2:    # f ready: c = f*c + i*g
                            fc_ = rga.tile([128, 128], F32, tag="fc")
                            nc.vector.tensor_mul(fc_[:], gate[2][:], c_prev[:])
                            c_new = rga.tile([128, 128], F32, tag="cn")
                            nc.vector.tensor_add(c_new[:], fc_[:], ig[:])
                            tc_ = rga.tile([128, 128], F32, tag="tc")
                            nc.scalar.activation(tc_[:], c_new[:], AF.Tanh)
                    nc.vector.tensor_mul(
                        Hk[:, :, t + 1, :],
                        gate[3][:].rearrange("p (k b) -> p k b", k=4),
                        tc_[:].rearrange("p (k b) -> p k b", k=4))
                    c_prev = c_new

            # fp8 twin of H for the FC (same column layout -> plain cast)
            nc.vector.tensor_copy(H8[:, :4096], H[:, :4096])
            nc.vector.tensor_copy(H8[:, 4096:], H[:, 4096:])

        # ===== phase D1: scores/softmax/ctx (batched over t) =====
        with ExitStack() as d1:
            dsb = d1.enter_context(tc.tile_pool(name="d1_sbuf", bufs=2))
            dps = d1.enter_context(
                tc.tile_pool(name="d1_psum", bufs=2, space="PSUM"))
            dst_ = d1.enter_context(tc.tile_pool(name="d1_stat", bufs=2))
            att_pool = d1.enter_context(tc.tile_pool(name="d1_att", bufs=1))
            enc_pool = d1.enter_context(tc.tile_pool(name="d1_enc", bufs=1))

            attnT_sb = att_pool.tile([128, 16 * T], BF16)  # attn.T 2b/tile

            for j in range(16):          # pairs of b
                scp = dps.tile([128, 64], F32, tag="sc")
                for half in range(2):
                    b = 2 * j + half
                    po = 64 * half
                    for k in range(4):
                        nc.tensor.matmul(
                            scp[po:po + T, :],
                            Hk[:, k, 1:64, b],
                            epT_sb[:, k * BL + b * L:
                                   k * BL + (b + 1) * L],
                            start=(k == 0), stop=(k == 3))
                # merged softmax over both halves (partitions 0-62, 64-126)
                att_f = dsb.tile([128, 64], F32, tag="af")
                attb = dsb.tile([128, 64], BF16, tag="ab")
                P2 = 64 + T
                nmx = dst_.tile([128, 1], F32, tag="nm")
                nc.vector.tensor_reduce(nmx[:P2, :], scp[:P2, :],
                                        axis=AX.X, op=ALU.max, negate=True)
                ssum = dst_.tile([128, 1], F32, tag="ss")
                nc.scalar.activation(att_f[:P2, :], scp[:P2, :],
                                     AF.Exp, bias=nmx[:P2, :],
                                     accum_out=ssum[:P2, :])
                rcp = dst_.tile([128, 1], F32, tag="rc")
                nc.vector.reciprocal(rcp[:P2, :], ssum[:P2, :])
                nc.vector.tensor_scalar_mul(attb[:P2, :], att_f[:P2, :],
                                            rcp[:P2, :])
                for half in range(2):
                    po = 64 * half
                    tpp = dps.tile([128, T], BF16, tag="tpa")
                    nc.tensor.transpose(tpp[po:po + L, :],
                                        attb[po:po + T, :L],
                                        idt[po:po + T, po:po + T])
                    nc.vector.tensor_copy(
                        attnT_sb[po:po + L, j * T:(j + 1) * T],
                        tpp[po:po + L, :])

            # ctx.T per b -> Gc8 (fp8 for the DoubleRow FC)
            for j in range(16):
                for half in range(2):
                    b = 2 * j + half
                    po = 64 * half
                    for mu in range(4):
                        ctp = dps.tile([128, T], F32, tag="ctx")
                        nc.tensor.matmul(
                            ctp[:, :],
                            enc_sb[po:po + L,
                                   j * U + mu * 128: j * U + (mu + 1) * 128],
                            attnT_sb[po:po + L, j * T:(j + 1) * T],
                            start=True, stop=True)
                        nc.vector.tensor_copy(Gc8k[:, mu, :, b], ctp[:, :])
        encs.close()

        # ======== phase D2: FC + stats + AllReduce + finalize ========
        with ExitStack() as d2:
            fps = d2.enter_context(
                tc.tile_pool(name="d2_psum", bufs=2, space="PSUM"))
            lgp = d2.enter_context(tc.tile_pool(name="d2_lg", bufs=1))
            scr = d2.enter_context(tc.tile_pool(name="d2_scr", bufs=2))
            sst = d2.enter_context(tc.tile_pool(name="d2_st", bufs=2))
            HALF = VS // 2            # 2000
            QSL = [(0, 512), (512, 512), (1024, 512), (1536, 464)]
            # all logits stay resident in SBUF as fp8 e3m4 (62.5KB/part)
            lg8 = lgp.tile([128, 16 * VS], FP8L)

            def fc_chunk(mi):
                r0 = mi * 128
                rows = min(128, TB - r0)
                ac = [None, None]
                for half in range(2):
                    fcp = fps.tile([128, HALF], F32, tag="fc")
                    for kp in range(4):
                        if kp < 2:
                            lhs = Gc8v[:, kp, :, r0:r0 + rows]
                        else:
                            lhs = H8v[:, kp - 2, :, B + r0: B + r0 + rows]
                        for off, w in QSL:
                            nc.tensor.matmul(
                                fcp[:rows, off:off + w],
                                lhs,
                                wfcv[:, kp, :, half * HALF + off:
                                     half * HALF + off + w],
                                start=(kp == 0), stop=(kp == 3),
                                perf_mode=PM.DoubleRow)
                    # bias add in PSUM (vector), then tanh -> fp8 store
                    nc.vector.tensor_add(
                        fcp[:rows, :], fcp[:rows, :],
                        bfc_bc[:rows, half * HALF:(half + 1) * HALF])
                    lsl = lg8[:rows, mi * VS + half * HALF:
                              mi * VS + (half + 1) * HALF]
                    nc.scalar.activation(lsl, fcp[:rows, :], AF.Tanh)
                    junk = scr.tile([128, HALF], FP8L, tag="jk")
                    acx = sst.tile([128, 1], F32, tag="ac")
                    nc.scalar.activation(junk[:rows, :], lsl, AF.Exp,
                                         bias=negone[:rows, :],
                                         accum_out=acx[:rows, :])
                    ac[half] = acx
                nc.vector.tensor_add(lsum_sb[:rows, mi:mi + 1],
                                     ac[0][:rows, :], ac[1][:rows, :])

            def allreduce_group(c0_, c1_):
                g = 0 if c0_ == 0 else 1
                nc.sync.dma_start(ccs_in[g][:], lsum_sb[:, c0_:c1_])
                nc.gpsimd.collective_compute(
                    "AllReduce", ALU.add,
                    replica_groups=[list(range(n_cores))],
                    ins=[ccs_in[g][:].opt()],
                    outs=[ccs_out[g][:].opt()])
                nc.sync.dma_start(sg_sb[:, c0_:c1_], ccs_out[g][:])
                lns = sst.tile([128, 16], F32, tag="ln")
                nc.scalar.activation(lns[:, c0_:c1_], sg_sb[:, c0_:c1_],
                                     AF.Ln)
                nc.vector.tensor_scalar(nlz_sb[:, c0_:c1_], lns[:, c0_:c1_],
                                        -1.0, -1.0, op0=ALU.mult, op1=ALU.add)

            def finalize_chunk(mi):
                r0 = mi * 128
                rows = min(128, TB - r0)
                of = scr.tile([128, VS], BF16, tag="of")
                nc.scalar.activation(
                    of[:rows, :HALF],
                    lg8[:rows, mi * VS: mi * VS + HALF],
                    AF.Identity, bias=nlz_sb[:rows, mi:mi + 1])
                nc.vector.tensor_scalar_add(
                    of[:rows, HALF:],
                    lg8[:rows, mi * VS + HALF: (mi + 1) * VS],
                    nlz_sb[:rows, mi:mi + 1])
                t0 = mi * 4
                for tl in range(rows // B):   # 4 (or 3) t's per chunk
                    nc.sync.dma_start(out[:, t0 + tl, :],
                                      of[tl * B:(tl + 1) * B, :])

            for mi in range(GA):
                fc_chunk(mi)
            if n_cores > 1:
                allreduce_group(0, GA)
            for mi in range(GA, 16):
                fc_chunk(mi)
            # finalize of group A overlaps group-B FC + its AllReduce
            if n_cores > 1:
                for mi in range(GA):
                    finalize_chunk(mi)
                allreduce_group(GA, 16)
            else:
                nc.vector.tensor_copy(sg_sb[:], lsum_sb[:])
                lns = sst.tile([128, 16], F32, tag="ln")
                nc.scalar.activation(lns[:], sg_sb[:], AF.Ln)
                nc.vector.tensor_scalar(nlz_sb[:], lns[:], -1.0, -1.0,
                                        op0=ALU.mult, op1=ALU.add)
                for mi in range(GA):
                    finalize_chunk(mi)
            for mi in range(GA, 16):
                finalize_chunk(mi)

    nc.compile()
    return nc


def _bf(x):
    return np.ascontiguousarray(
        np.asarray(x, np.float32).astype(ml_dtypes.bfloat16))


def _f8(x):
    return np.ascontiguousarray(
        np.asarray(x, np.float32).astype(ml_dtypes.float8_e4m3))


def prep_inputs(target, encoder_outputs, enc_h0, enc_c0, emb, W_x, W_h,
                b_lstm, Wa, W_fc, b_fc, n_cores=NC):
    """Host-side layout prep + per-core sharding."""
    tgt = np.asarray(target).astype(np.int32)
    tidx = np.ascontiguousarray(tgt[:, :T].T.reshape(TB, 1))  # t-major rows
    enc = np.asarray(encoder_outputs, np.float32)
    # permute gate columns [i,f,g,o] -> [g,i,f,o]: the kernel computes each
    # gate as soon as its matmul chunk group lands (g first, o last)
    gperm = np.r_[2 * U:3 * U, 0:U, U:2 * U, 3 * U:4 * U]
    wx = np.asarray(W_x, np.float32)[:, gperm]
    # wx8: [p, e2, m] fp8 (contraction pair = the two E chunks of 128)
    wx8 = wx.reshape(2, 128, G4U).transpose(1, 0, 2).reshape(128, 2 * G4U)
    blp = np.asarray(b_lstm, np.float32)[gperm].reshape(16, 128).T
    W_h = np.asarray(W_h, np.float32)[:, gperm]
    common = {
        "tidx": tidx,
        "emb_bf": _bf(emb),
        "wx8": _f8(wx8),
        "blp": np.ascontiguousarray(blp),
        "wh_bf": _bf(W_h),
        "enc_bf": _bf(enc),
        "enct_bf": _bf(enc.transpose(2, 0, 1).reshape(U, BL)),
        "wa_bf": _bf(Wa),
        "h0": np.ascontiguousarray(np.asarray(enc_h0, np.float32)),
        "c0": np.ascontiguousarray(np.asarray(enc_c0, np.float32)),
    }
    wfc = np.asarray(W_fc, np.float32)
    bfc = np.asarray(b_fc, np.float32)
    in_maps = []
    for c in range(n_cores):
        m = dict(common)
        shard = wfc[:, c * VS:(c + 1) * VS]           # [1024, VS]
        # [p, kp, k2, v] pair-interleaved for DoubleRow
        m["wfc8"] = _f8(shard.reshape(4, 2, 128, VS)
                        .transpose(2, 0, 1, 3).reshape(128, 8 * VS))
        m["bfc_bf"] = _bf(bfc[c * VS:(c + 1) * VS].reshape(1, VS))
        in_maps.append(m)
    return in_maps


def kernel(**inputs):
    if "nc" not in _CACHE:
        _CACHE["nc"] = build(NC)
    nc = _CACHE["nc"]
    in_maps = prep_inputs(**inputs, n_cores=NC)
    res = run_bass_kernel_spmd(nc, in_maps, list(range(NC)))
    shards = [np.asarray(res.results[c]["out"], np.float32)
              for c in range(NC)]
    return np.concatenate(shards, axis=-1)
